# revision 1
# baseline (speedup 1.0000x reference)
"""AverageSpanExtractor Trainium2 kernel.

Math: out[b, n, :] = mean(seq[b, start_n:end_n, :]) * mask[b, n]

Strategy (per core; data-parallel over batch across 8 cores):
  1. Load seq [S=2048, D=512] f32 into SBUF.
  2. Build exclusive prefix-sum table E[2049, 512] in DRAM:
       - per 128-token block: in-block inclusive cumsum via PE matmul with an
         upper-triangular ones matrix (U.T @ X), plus the block offset
         broadcast into the same PSUM accumulation via a one-hot selector
         matmul against a [16, 512] table of running block offsets.
       - block offsets come from block totals (all-ones matmul, one row
         copied per block) run through a tiny strict-upper-triangular matmul.
  3. Gather E[end_n] and E[start_n] rows for all 1024 spans with
     gpsimd.dma_gather (2048 rows x 2KiB), subtract on DVE, scale by
     mask/width on ACT, store.
"""

import numpy as np

import concourse.bacc as bacc
import concourse.bass as bass
import concourse.tile as tile
from concourse import mybir
from concourse.bass import AP
from concourse.library_config import mlp
from concourse.masks import make_upper_triangular
from concourse.tile_rust import add_dep_helper

# Problem shape (hardcoded per contract).
B, S, D, N = 8, 2048, 512, 1024
NBLK = S // 128          # 16 token blocks
NTILE = N // 128         # 8 span tiles
NGATHER = 4              # gather instructions (2 span tiles each)
TBL_ROWS = S + 1         # 2049

F32 = mybir.dt.float32
F32R = mybir.dt.float32r
I32 = mybir.dt.int32
I16 = mybir.dt.int16

BF16 = mybir.dt.bfloat16
F16 = mybir.dt.float16

# Matmuls run in 16-bit: the sequence data as fp16 (11 mantissa bits), the
# block-offset path as an exact bf16 hi+lo pair. 16-bit is the only matmul
# path that runs at 1 cycle/row AND warms the PE clock gate (fp32/fp32r go
# through the transpose-mode path the HAM activity monitor ignores, pinning
# the PE at 1.2 GHz).


def _mm(ap: AP, dt) -> AP:
    return ap.bitcast(dt) if ap.dtype != dt else ap


def build_kernel_body(tc: tile.TileContext, seq: AP, spans: AP, maskw: AP,
                      out: AP, ctx, dbg=None):
    nc = tc.nc
    sbuf = ctx.enter_context(tc.tile_pool(name="sbuf", bufs=1))
    const = ctx.enter_context(tc.tile_pool(name="const", bufs=1))
    epool = ctx.enter_context(tc.tile_pool(name="epool", bufs=5))
    gpool = ctx.enter_context(tc.tile_pool(name="gpool", bufs=1))
    dpool = ctx.enter_context(tc.tile_pool(name="dpool", bufs=3))
    opool = ctx.enter_context(tc.tile_pool(name="opool", bufs=3))
    psum_tot = ctx.enter_context(tc.tile_pool(name="ptot", bufs=3, space="PSUM"))
    psum_e = ctx.enter_context(tc.tile_pool(name="pe", bufs=4, space="PSUM"))
    psum_off = ctx.enter_context(tc.tile_pool(name="poff", bufs=1, space="PSUM"))
    dram = ctx.enter_context(tc.tile_pool(name="dram", bufs=1, space="DRAM"))

    table = dram.tile([TBL_ROWS, D], F32)

    # ---------------- constants (first: DVE casts lead the queue) ----------------
    # Build in f32 (memset/affine_select), then cast on DVE (0/1 exact).
    u_tri_f = const.tile([128, 128], F32, tag="u_tri_f")
    make_upper_triangular(nc, u_tri_f[:], val=1.0, diag=True)
    u_tri = const.tile([128, 128], F16, tag="u_tri")
    nc.vector.tensor_copy(u_tri[:], u_tri_f[:])

    u16s_f = const.tile([16, 16], F32, tag="u16s_f")
    make_upper_triangular(nc, u16s_f[:], val=1.0, diag=False)
    u16s = const.tile([16, 16], BF16, tag="u16s")
    nc.vector.tensor_copy(u16s[:], u16s_f[:])

    zrow = const.tile([1, D], F32, tag="zrow")
    nc.gpsimd.memset(zrow[:], 0.0)
    zrow_store = nc.sync.dma_start(table[0:1, :], zrow[:])



    # ---------------- index / scale staging (Sync queue, before loads) -------------
    # spans int32 [N, 2] = (start, end);  maskw int32 [N]
    #
    # Gather index list (linear order i within gather t of 512 idxs):
    #   i in [0,256):   end of span 256t + i
    #   i in [256,512): start of span 256t + (i - 256)
    # dma_gather reads idxs[p, c] = list[c*16 + p%16], so list position i sits
    # at column i//16, partition i%16 -> global column c = 32t + i//16.
    a32 = sbuf.tile([16, 128], I32, tag="a32")
    for t in range(NGATHER):
        # ends: dst cols 32t+u (u<16); src element = spans[256t + 16u + p, 1]
        nc.sync.dma_start(
            a32[:, 32 * t:32 * t + 16],
            AP(spans.tensor, 512 * t + 1, [[2, 16], [32, 16]]))
        # starts: dst cols 32t+16+u; src = spans[256t + 16u + p, 0]
        nc.sync.dma_start(
            a32[:, 32 * t + 16:32 * t + 32],
            AP(spans.tensor, 512 * t, [[2, 16], [32, 16]]))


    idx16 = sbuf.tile([128, 128], I16, tag="idx16")
    nc.vector.tensor_copy(idx16[0:16, :], a32[:])
    # replicate 16-partition wrap across all 128 partitions (8 Q7 cores)
    nc.scalar.dma_start(idx16[16:32, :], idx16[0:16, :])
    nc.scalar.dma_start(idx16[32:64, :], idx16[0:32, :])
    nc.scalar.dma_start(idx16[64:128, :], idx16[0:64, :])

    # ------- phase 1a: seq loads first (Sync HWDGE), cast to fp16 on DVE ---
    # fp16 keeps 11 mantissa bits (~2.4e-4 per-term); every sum reads the
    # SAME fp16 values so prefix differences stay consistent (~1e-4 relative
    # on the means).
    xbig = sbuf.tile([128, NBLK, D], F32, tag="xbig")
    xf = sbuf.tile([128, NBLK, D], F16, tag="xf")
    for q in range(NBLK // 4):
        sl = (slice(None), slice(4 * q, 4 * q + 4), slice(None))
        nc.sync.dma_start(
            xbig[sl],
            seq[512 * q:512 * (q + 1), :].rearrange("(j p) d -> p j d", p=128))
        nc.vector.tensor_copy(xf[sl], xbig[sl])

    # ------- prepare gather descriptors early (idle Q7), trigger later -----
    # Traced BEFORE any table store so the preps carry no RAW dep on the
    # table; the trigger gets explicit deps on the stores instead.
    # load the gather ucode library now: the reload blocks the Pool engine
    # ~10us, so it runs after the constants the first matmuls depend on.
    nc.gpsimd.load_library(mlp)
    gsems = [ctx.enter_context(nc.semaphore(f"gsem{t}"))
             for t in range(NGATHER)]
    gts = []
    for t in range(NGATHER):
        g_t = gpool.tile([128, 4, D], F32, tag=f"g{t}")
        nc.gpsimd.dma_gather(
            out_ap=g_t[:], in_ap=table[:], idxs_ap=idx16[:, 32 * t:32 * t + 32],
            num_idxs=512, num_idxs_reg=512, elem_size=D,
            prepare_only=True, sem=gsems[t])
        gts.append(g_t)

    # sel64[b]: [64, 128] with ones at rows k==b and k==32+b — selects the
    # bf16 hi (partitions 0:16) and lo (partitions 32:48) offset rows of o2
    # and broadcasts their sum across all 128 output partitions.
    sels = []
    for b in range(NBLK):
        sel_f = const.tile([64, 128], F32, tag=f"self{b}")
        nc.gpsimd.memset(sel_f[:], 0.0)
        nc.gpsimd.affine_select(
            out=sel_f[:], in_=sel_f[:], compare_op=mybir.AluOpType.not_equal,
            fill=1.0, base=-b, pattern=[[0, 128]], channel_multiplier=1)
        nc.gpsimd.affine_select(
            out=sel_f[:], in_=sel_f[:], compare_op=mybir.AluOpType.not_equal,
            fill=1.0, base=-(32 + b), pattern=[[0, 128]], channel_multiplier=1)
        sel_b = const.tile([64, 128], BF16, tag=f"selb{b}")
        nc.vector.tensor_copy(sel_b[:], sel_f[:])
        sels.append(sel_b)


    # per-span scale = mask / width, laid out [p, j] for span n = j*128 + p
    st_pj = sbuf.tile([128, NTILE], I32, tag="st_pj")
    en_pj = sbuf.tile([128, NTILE], I32, tag="en_pj")
    mk_pj = sbuf.tile([128, NTILE], I32, tag="mk_pj")
    nc.sync.dma_start(st_pj[:], AP(spans.tensor, 0, [[2, 128], [256, NTILE]]))
    nc.sync.dma_start(en_pj[:], AP(spans.tensor, 1, [[2, 128], [256, NTILE]]))
    nc.sync.dma_start(mk_pj[:], AP(maskw.tensor, 0, [[1, 128], [128, NTILE]]))

    w_i = sbuf.tile([128, NTILE], I32, tag="w_i")
    nc.vector.tensor_tensor(out=w_i[:], in0=en_pj[:], in1=st_pj[:],
                            op=mybir.AluOpType.subtract)
    w_f = sbuf.tile([128, NTILE], F32, tag="w_f")
    nc.vector.tensor_copy(w_f[:], w_i[:])
    r_f = sbuf.tile([128, NTILE], F32, tag="r_f")
    nc.vector.reciprocal(r_f[:], w_f[:])
    m_f = sbuf.tile([128, NTILE], F32, tag="m_f")
    nc.vector.tensor_copy(m_f[:], mk_pj[:])
    scale = sbuf.tile([128, NTILE], F32, tag="scale")
    nc.vector.tensor_tensor(out=scale[:], in0=r_f[:], in1=m_f[:],
                            op=mybir.AluOpType.mult)

    # ---------------- phase 1: in-block cumsums (no offsets yet) ----------
    # L_b = u_tri.T @ xf_b; its last row (partition 127) IS the block total,
    # so no separate totals pass is needed.
    lbig = sbuf.tile([128, NBLK, D], F32, tag="lbig")
    for b in range(NBLK):
        pl = psum_e.tile([128, D], F32, tag="pe")
        nc.tensor.matmul(out=pl[:], lhsT=u_tri[:],
                         rhs=xf[:, b, :], start=True, stop=True)
        nc.scalar.copy(lbig[:, b, :], pl[:])

    # T[16, 512] <- block totals (lbig partition 127, slots 0..14), then
    # running offsets Off = strict_upper(U16).T @ T.
    # NB: keep the source AP's partition dim honest (partition 127 only) —
    # reshaping free extents into the AP's partition slot confuses Tile's
    # dep tracking and the DMA races ahead of the producers.
    t16 = sbuf.tile([16, D], F32, tag="t16")
    nc.vector.memset(t16[:], 0.0)
    nc.sync.dma_start(t16[0:NBLK - 1, :], lbig[127:128, 0:NBLK - 1, :])

    # split t16 into bf16 hi/lo, run the tiny strict-upper matmul in bf16,
    # then pack the offsets as bf16 hi/lo into o2 rows 0:16 / 32:48.
    th = sbuf.tile([16, D], BF16, tag="th")
    nc.vector.tensor_copy(th[:], t16[:])
    tl = sbuf.tile([16, D], BF16, tag="tl")
    nc.vector.tensor_tensor(out=tl[:], in0=t16[:], in1=th[:],
                            op=mybir.AluOpType.subtract)
    poff = psum_off.tile([16, D], F32, tag="poff")
    nc.tensor.matmul(out=poff[:], lhsT=u16s[:], rhs=th[:], start=True, stop=False)
    nc.tensor.matmul(out=poff[:], lhsT=u16s[:], rhs=tl[:], start=False, stop=True)
    off16 = sbuf.tile([16, D], F32, tag="off16")
    nc.vector.tensor_copy(off16[:], poff[:])
    o2 = sbuf.tile([64, D], BF16, tag="o2")
    nc.vector.memset(o2[:], 0.0)
    nc.vector.tensor_copy(o2[0:16, :], off16[:])
    nc.vector.tensor_tensor(out=o2[32:48, :], in0=off16[:], in1=o2[0:16, :],
                            op=mybir.AluOpType.subtract)

    # ---------------- phase 1b: add offsets, store table -------------------
    store_insts = []
    for b in range(NBLK):
        p2 = psum_tot.tile([128, D], F32, tag="p2")
        nc.tensor.matmul(out=p2[:], lhsT=sels[b][:], rhs=o2[:],
                         start=True, stop=True)
        e_t = epool.tile([128, D], F32, tag="e")
        nc.vector.tensor_tensor(out=e_t[:], in0=lbig[:, b, :], in1=p2[:],
                                op=mybir.AluOpType.add)
        store_insts.append(
            nc.sync.dma_start(table[1 + 128 * b:1 + 128 * (b + 1), :], e_t[:]))
        if dbg is not None:
            nc.sync.dma_start(dbg["tbl"][1 + 128 * b:1 + 128 * (b + 1), :], e_t[:])

    if dbg is not None:
        nc.sync.dma_start(dbg["tbl"][0:1, :], zrow[:])
        nc.sync.dma_start(dbg["idx"][:], idx16[:])
        nc.sync.dma_start(dbg["scale"][:], scale[:])
        nc.sync.dma_start(dbg["xbig"][:], xbig[:])
        nc.sync.dma_start(dbg["t16"][:], t16[:])
        nc.sync.dma_start(dbg["off16"][:], off16[:])

    # ---------------- phase 2: fire prepared gathers, combine --------------
    trig = nc.gpsimd.trigger_dma(count=None)
    for st in store_insts:
        add_dep_helper(trig.ins, st.ins, sync=True, reason="gather transfers read table")
    add_dep_helper(trig.ins, zrow_store.ins, sync=True, reason="gather reads table row 0")

    for t in range(NGATHER):
        g_t = gts[t]
        if dbg is not None:
            gd = nc.sync.dma_start(dbg["g"][:, 4 * t:4 * t + 4, :], g_t[:])
            gd._wait_ge(gsems[t], 16)
            add_dep_helper(gd.ins, trig.ins, sync=False,
                           reason="consume after trigger")
        for k in range(2):
            j = 2 * t + k
            d_t = dpool.tile([128, D], F32, tag="d")
            tt = nc.vector.tensor_tensor(out=d_t[:], in0=g_t[:, k, :],
                                         in1=g_t[:, 2 + k, :],
                                         op=mybir.AluOpType.subtract)
            tt._wait_ge(gsems[t], 16)
            add_dep_helper(tt.ins, trig.ins, sync=False,
                           reason="consume after trigger")
            o_t = opool.tile([128, D], F32, tag="o")
            nc.scalar.mul(o_t[:], d_t[:], scale[:, j:j + 1])
            nc.sync.dma_start(out[128 * j:128 * (j + 1), :], o_t[:])


def build_nc(debug_taps=False):
    nc = bacc.Bacc("TRN2", target_bir_lowering=False, debug=False,
                   dynamic_dma_scratch_size=2 ** 16)
    seq = nc.dram_tensor("seq", [S, D], F32, kind="ExternalInput")
    spans = nc.dram_tensor("spans", [N, 2], I32, kind="ExternalInput")
    maskw = nc.dram_tensor("maskw", [N], I32, kind="ExternalInput")
    out = nc.dram_tensor("out", [N, D], F32, kind="ExternalOutput")
    dbg = None
    if debug_taps:
        dbg = {
            "tbl": nc.dram_tensor("dbg_tbl", [TBL_ROWS, D], F32,
                                  kind="ExternalOutput").ap(),
            "idx": nc.dram_tensor("dbg_idx", [128, 128], I16,
                                  kind="ExternalOutput").ap(),
            "scale": nc.dram_tensor("dbg_scale", [128, NTILE], F32,
                                    kind="ExternalOutput").ap(),
            "g": nc.dram_tensor("dbg_g", [128, 4 * NGATHER, D], F32,
                                kind="ExternalOutput").ap(),
            "xbig": nc.dram_tensor("dbg_xbig", [128, NBLK, D], F32,
                                   kind="ExternalOutput").ap(),
            "t16": nc.dram_tensor("dbg_t16", [16, D], F32,
                                  kind="ExternalOutput").ap(),
            "off16": nc.dram_tensor("dbg_off16", [16, D], F32,
                                    kind="ExternalOutput").ap(),
        }
    from contextlib import ExitStack
    with tile.TileContext(nc) as tc:
        with ExitStack() as ctx:
            build_kernel_body(tc, seq.ap(), spans.ap(), maskw.ap(), out.ap(),
                              ctx, dbg=dbg)
    nc.compile()
    return nc


_NC_CACHE = None


def kernel(sequence_tensor: np.ndarray, span_indices: np.ndarray,
           span_indices_mask: np.ndarray) -> np.ndarray:
    global _NC_CACHE
    from concourse.bass_utils import run_bass_kernel_spmd

    if _NC_CACHE is None:
        _NC_CACHE = build_nc()
    nc = _NC_CACHE

    spans_i32 = np.ascontiguousarray(np.asarray(span_indices).astype(np.int32))
    mask_i32 = np.ascontiguousarray(np.asarray(span_indices_mask).astype(np.int32))
    seq_f32 = np.ascontiguousarray(sequence_tensor, dtype=np.float32)

    in_maps = [
        {"seq": seq_f32[b], "spans": spans_i32[b], "maskw": mask_i32[b]}
        for b in range(B)
    ]
    res = run_bass_kernel_spmd(nc, in_maps, core_ids=list(range(B)))
    return np.stack([r["out"] for r in res.results], axis=0)



# revision 3
# speedup vs baseline: 1.5194x; 1.5194x over previous
"""AverageSpanExtractor Trainium2 kernel.

Math: out[b, n, :] = mean(seq[b, start_n:end_n, :]) * mask[b, n]

Strategy (per core; data-parallel over batch across 8 cores):
  1. Load seq [S=2048, D=512] f32, cast fp16.
  2. Per 128-token block: in-block inclusive cumsum via PE matmul with an
     upper-triangular ones matrix; store the UNOFFSET cumsums straight to a
     DRAM table [2048, 512] fp16 (token i = sum seq[0..i], no running block
     offsets on the store path).
  3. Gather token end-1 and token max(start-1, 0) for all spans with
     gpsimd.dma_gather on 4 PARALLEL SWDGE queues (fp16 rows, 1KiB each).
  4. Post-gather correction: the missing block offsets (and the start==0
     edge case) are a tiny matmul C_j = A_j.T @ T17, where A_j [17, 128]
     is a host-computed 0/1 selector and T17 holds the 16 block totals
     (table row 127 of each block) plus token 0 (= seq row 0).
     out_j = (G_end - G_start) * scale + C_j * ... — scale folded on host
     into A, so out_j = (G_end - G_start)*scale + Cs_j.
  5. All index-derived tensors (gather idx list, per-span scale, A
     selectors) are precomputed on the host from the int span inputs.
"""

import numpy as np

import concourse.bacc as bacc
import concourse.bass as bass
import concourse.tile as tile
from concourse import mybir
from concourse.bass import AP
from concourse.library_config import mlp
from concourse.masks import make_upper_triangular
from concourse.tile_rust import add_dep_helper

# Problem shape (hardcoded per contract).
B, S, D, N = 8, 2048, 512, 1024
NBLK = S // 128          # 16 token blocks
NTILE = N // 128         # 8 span tiles
NGATHER = 4              # gather instructions (2 span tiles each), 1 queue each
NQUAD = 4                # seq load / table store granularity: 4 blocks

F32 = mybir.dt.float32
I32 = mybir.dt.int32
I16 = mybir.dt.int16
F16 = mybir.dt.float16


def build_kernel_body(tc: tile.TileContext, seq: AP, idx16_in: AP, scale_in: AP,
                      asel_in: AP, out: AP, ctx):
    nc = tc.nc
    sbuf = ctx.enter_context(tc.tile_pool(name="sbuf", bufs=1))
    const = ctx.enter_context(tc.tile_pool(name="const", bufs=1))
    gpool = ctx.enter_context(tc.tile_pool(name="gpool", bufs=1))
    dpool = ctx.enter_context(tc.tile_pool(name="dpool", bufs=3))
    opool = ctx.enter_context(tc.tile_pool(name="opool", bufs=3))
    psum_e = ctx.enter_context(tc.tile_pool(name="pe", bufs=4, space="PSUM"))
    psum_c = ctx.enter_context(tc.tile_pool(name="pc", bufs=3, space="PSUM"))
    dram = ctx.enter_context(tc.tile_pool(name="dram", bufs=1, space="DRAM"))

    table = dram.tile([S, D], F16)

    # ---------------- constants (DVE cast leads the queue) -----------------
    u_tri_f = const.tile([128, 128], F32, tag="u_tri_f")
    make_upper_triangular(nc, u_tri_f[:], val=1.0, diag=True)
    u_tri = const.tile([128, 128], F16, tag="u_tri")
    nc.vector.tensor_copy(u_tri[:], u_tri_f[:])

    # ---------------- host-precomputed index tensors (ACT queue) -----------
    idx16 = sbuf.tile([128, 128], I16, tag="idx16")
    nc.scalar.dma_start(idx16[:], idx16_in)
    scale = sbuf.tile([128, NTILE], F32, tag="scale")
    nc.scalar.dma_start(scale[:], scale_in)
    asel = sbuf.tile([17, N], F16, tag="asel")
    nc.scalar.dma_start(asel[:], asel_in)

    # ---------------- seq loads (Sync HWDGE), cast to fp16 on DVE ----------
    xbig = sbuf.tile([128, NBLK, D], F32, tag="xbig")
    xf = sbuf.tile([128, NBLK, D], F16, tag="xf")
    for q in range(NQUAD):
        sl = (slice(None), slice(4 * q, 4 * q + 4), slice(None))
        nc.sync.dma_start(
            xbig[sl],
            seq[512 * q:512 * (q + 1), :].rearrange("(j p) d -> p j d", p=128))
        nc.vector.tensor_copy(xf[sl], xbig[sl])

    # ------- prepare gathers early (idle Q7 cores), trigger later ----------
    # Traced BEFORE any table store so the preps carry no RAW dep on the
    # table; each trigger gets explicit deps on the stores instead.
    nc.gpsimd.load_library(mlp)
    gsems = [ctx.enter_context(nc.semaphore(f"gsem{t}"))
             for t in range(NGATHER)]
    gts = []
    for t in range(NGATHER):
        g_t = gpool.tile([128, 4, D], F16, tag=f"g{t}")
        nc.gpsimd.dma_gather(
            out_ap=g_t[:], in_ap=table[:], idxs_ap=idx16[:, 32 * t:32 * t + 32],
            num_idxs=512, num_idxs_reg=512, elem_size=D,
            prepare_only=True, sem=gsems[t], queue_num=t)
        gts.append(g_t)

    # ---------------- in-block cumsums -> fp16 table stores ----------------
    # L_b = u_tri.T @ xf_b (inclusive cumsum); ACT casts PSUM f32 -> fp16
    # into ebig; one store DMA per quad of blocks.
    ebig = sbuf.tile([128, NBLK, D], F16, tag="ebig")
    store_insts = []
    for q in range(NQUAD):
        for bb in range(4):
            b = 4 * q + bb
            pl = psum_e.tile([128, D], F32, tag="pe")
            nc.tensor.matmul(out=pl[:], lhsT=u_tri[:],
                             rhs=xf[:, b, :], start=True, stop=True)
            nc.scalar.copy(ebig[:, b, :], pl[:])
        store_insts.append(nc.sync.dma_start(
            table[512 * q:512 * (q + 1), :].rearrange("(j p) d -> p j d", p=128),
            ebig[:, 4 * q:4 * q + 4, :]))

    # T17: rows 0..15 = block totals (ebig partition 127), row 16 = token 0
    # (= seq row 0 = ebig[0, 0, :]) for the start==0 redirect.
    t17 = sbuf.tile([17, D], F16, tag="t17")
    nc.sync.dma_start(t17[0:NBLK, :], ebig[127:128, 0:NBLK, :])
    nc.sync.dma_start(t17[NBLK:NBLK + 1, :], ebig[0:1, 0, :])

    # ---------------- fire prepared gathers (4 parallel queues) ------------
    trigs = []
    for t in range(NGATHER):
        trig = nc.gpsimd.trigger_dma(count=None, queue_num=t)
        for st in store_insts:
            add_dep_helper(trig.ins, st.ins, sync=True,
                           reason="gather reads whole table")
        trigs.append(trig)

    # ---------------- combine: (G_end - G_start)*scale + A.T @ T17 ---------
    for t in range(NGATHER):
        g_t = gts[t]
        for k in range(2):
            j = 2 * t + k
            pc = psum_c.tile([128, D], F32, tag="pc")
            nc.tensor.matmul(out=pc[:], lhsT=asel[:, 128 * j:128 * (j + 1)],
                             rhs=t17[:], start=True, stop=True)
            d_t = dpool.tile([128, D], F32, tag="d")
            tt = nc.vector.tensor_tensor(out=d_t[:], in0=g_t[:, k, :],
                                         in1=g_t[:, 2 + k, :],
                                         op=mybir.AluOpType.subtract)
            tt._wait_ge(gsems[t], 16)
            add_dep_helper(tt.ins, trigs[t].ins, sync=False,
                           reason="consume after trigger")
            m_t = dpool.tile([128, D], F32, tag="m")
            nc.scalar.mul(m_t[:], d_t[:], scale[:, j:j + 1])
            o_t = opool.tile([128, D], F32, tag="o")
            nc.vector.tensor_tensor(out=o_t[:], in0=m_t[:], in1=pc[:],
                                    op=mybir.AluOpType.add)
            nc.sync.dma_start(out[128 * j:128 * (j + 1), :], o_t[:])


def build_nc():
    nc = bacc.Bacc("TRN2", target_bir_lowering=False, debug=False,
                   dynamic_dma_scratch_size=2 ** 16, num_swdge_queues=4)
    seq = nc.dram_tensor("seq", [S, D], F32, kind="ExternalInput")
    idx16 = nc.dram_tensor("idx16", [128, 128], I16, kind="ExternalInput")
    scale = nc.dram_tensor("scale", [128, NTILE], F32, kind="ExternalInput")
    asel = nc.dram_tensor("asel", [17, N], F16, kind="ExternalInput")
    out = nc.dram_tensor("out", [N, D], F32, kind="ExternalOutput")
    from contextlib import ExitStack
    with tile.TileContext(nc) as tc:
        with ExitStack() as ctx:
            build_kernel_body(tc, seq.ap(), idx16.ap(), scale.ap(), asel.ap(),
                              out.ap(), ctx)
    nc.compile()
    return nc


def host_precompute(span_indices: np.ndarray, span_indices_mask: np.ndarray):
    """Index-only preprocessing: gather idx list, per-span scale, offset
    selectors. Returns per-batch dicts of device input arrays."""
    spans = np.asarray(span_indices).astype(np.int64)      # [B, N, 2]
    mask = np.asarray(span_indices_mask).astype(np.int64)  # [B, N]
    starts = spans[..., 0]
    ends = spans[..., 1]
    widths = ends - starts                                  # >= 1

    # Gather token ids: token i holds sum seq[0..i]; E[e] = token e-1,
    # E[s] = token s-1, with s == 0 redirected to token 0 and compensated
    # via asel row 16 (+ token-0 value = seq row 0).
    tok_end = (ends - 1).astype(np.int64)                   # [B, N] in [0, S)
    tok_start = np.maximum(starts - 1, 0).astype(np.int64)

    # idx16[p, 32t + c] = list_t[c*16 + p%16];
    # list_t = [ends of spans 256t..256t+256) ++ starts of same]
    idx16 = np.empty((B, 128, 128), dtype=np.int16)
    for t in range(4):
        sl = slice(256 * t, 256 * t + 256)
        lst = np.concatenate([tok_end[:, sl], tok_start[:, sl]], axis=1)  # [B,512]
        wrapped = lst.reshape(B, 32, 16)                    # [B, c, p%16]
        block = np.transpose(wrapped, (0, 2, 1))            # [B, 16, 32]
        idx16[:, :, 32 * t:32 * t + 32] = np.tile(block, (1, 8, 1))

    # scale[p, j] = mask_n / width_n for n = 128j + p
    scale = (mask.astype(np.float32) /
             widths.astype(np.float32)).reshape(B, NTILE, 128)
    scale = np.ascontiguousarray(np.transpose(scale, (0, 2, 1)))  # [B,128,8]

    # asel[k, n]: correction selector.  C_n = sum_k asel[k, n] * T17[k]
    #   k < 16:  [k < blkE] - [k < blkS]   (blkS term dropped when start==0)
    #   k == 16: [start == 0]              (adds token-0 value = seq row 0)
    blk_e = tok_end // 128                                  # [B, N]
    blk_s = tok_start // 128
    ks = np.arange(16).reshape(1, 16, 1)
    a_e = (ks < blk_e[:, None, :])
    a_s = (ks < blk_s[:, None, :]) & (starts[:, None, :] > 0)
    # out = (d + C_raw) * s  ==  d*s + C  with the per-span scale s folded
    # into asel here (C = C_raw * s), so the device can scale d on ACT and
    # add the PSUM correction on DVE without an extra pass.
    s_n = (mask.astype(np.float32) / widths.astype(np.float32))[:, None, :]
    asel = np.zeros((B, 17, N), dtype=np.float32)
    asel[:, :16, :] = a_e.astype(np.float32) - a_s.astype(np.float32)
    asel[:, 16, :] = (starts == 0).astype(np.float32)
    asel = (asel * s_n).astype(np.float16)

    return [{"idx16": np.ascontiguousarray(idx16[b]),
             "scale": np.ascontiguousarray(scale[b]),
             "asel": np.ascontiguousarray(asel[b])} for b in range(B)]


def make_in_maps(sequence_tensor, span_indices, span_indices_mask):
    seq_f32 = np.ascontiguousarray(sequence_tensor, dtype=np.float32)
    host = host_precompute(span_indices, span_indices_mask)
    return [{"seq": seq_f32[b], **host[b]} for b in range(B)]


_NC_CACHE = None


def kernel(sequence_tensor: np.ndarray, span_indices: np.ndarray,
           span_indices_mask: np.ndarray) -> np.ndarray:
    global _NC_CACHE
    from concourse.bass_utils import run_bass_kernel_spmd

    if _NC_CACHE is None:
        _NC_CACHE = build_nc()
    nc = _NC_CACHE

    in_maps = make_in_maps(sequence_tensor, span_indices, span_indices_mask)
    res = run_bass_kernel_spmd(nc, in_maps, core_ids=list(range(B)))
    return np.stack([r["out"] for r in res.results], axis=0)
